# revision 1
# baseline (speedup 1.0000x reference)
"""Multi-head causal attention (B=2, S=2048, D=1024, H=16) on 8 trn2 cores.

Sharding: core c -> (batch b = c//4, head-group g = c%4, 4 heads each).
Data-parallel over B, tensor-parallel over heads. Each core computes a
partial output projection [S, D]; the host sums the 4 partials per batch
and adds b_out.

All matmul operands are bf16 (PSUM accumulation stays f32): same PE rate
as f32r at 1 cycle/row (and no f32r 4x penalty on narrow diagonal
blocks), half the DMA traffic and SBUF footprint of f32. The freed SBUF
double-buffers qkT / v_aug / valuesT / xt across repeat iterations, and
the repeat loop is 2x-unrolled so consecutive bodies ping-pong buffers
and overlap through plain dataflow deps (stage A of body k+1 runs during
stage B/C of body k); staggered_reset avoids the all-engine barrier at
the For_i back-edge. fp8/DoubleRow was evaluated and rejected: e4m3
quantization of x/W/v/values costs 2.6-3.3% max-rel error each against
the 2e-2 budget.

Device kernel per core:
  A) qkT[f=512, s=2048] = (x @ Wqk)^T (bf16, qk bias folded into the
     PSUM->SBUF copy as a DVE tensor_scalar_add with a [128,1] per-
     partition bias column) and v_aug[s, 4, hd+1] = x @ Wv + bv (ones
     col appended). Both x halves DMA'd up front (xt pool bufs=2);
     half-1 compute items drip into B(qmb0/1) as PE filler.
  B) per head h, per 512-wide query block qmb: causal flash attention in
     the scores-TRANSPOSED layout: sT[k,q] = K @ Q^T so that attn@V is
     lhsT=v_blk[s,hd+1] (ones col -> softmax denominators in row 64 of
     PSUM), rhs=expT[k,q]. No on-chip transposes. Causal mask applied on
     the PE (psum += I.T @ trimask, bf16). sc->exp->av chains are
     software-pipelined (av lags by `lag` links) over a rotating PSUM
     pool so cross-engine semaphore wake latency is hidden.
  C) out_partial[s, 1024] = values^T.T @ W_out, bf16 partials DMA'd to
     DRAM; host accumulates in f32.
"""
import math
import numpy as np
import ml_dtypes

import concourse.bass as bass
import concourse.mybir as mybir
import concourse.tile as tile
from concourse import bacc
from concourse.bass_utils import run_bass_kernel_spmd

N_CORES = 8
B, S, D = 2, 2048, 1024
H = 16                    # total heads
HL = 4                    # heads per core
HD = 64                   # head dim
FQK = 2 * HL * HD         # 512 local q+k features
FV = HL * HD              # 256 local v features
SCALE = 1.0 / math.sqrt(HD)
NEG = -1e9

QMB = 512                 # query macro-block
KB = 128                  # key block
N_QMB = S // QMB          # 4
N_KB = S // KB            # 16

F32 = mybir.dt.float32
F32R = mybir.dt.float32r
BF16 = mybir.dt.bfloat16
NP_BF16 = ml_dtypes.bfloat16


def build_kernel(repeat: int = 1, stages: str = "ABC",
                 bmode: str = "full", pairw: int = 2, wave: int = 2,
                 sc_bufs: int = 3, av_bufs: int = 2, exp_bufs: int = 8,
                 lag: int = 2, fullexp: bool = False,
                 fill_first: bool = False, unroll: int = 2,
                 staggered: bool = True):
    assert pairw == 2 and wave == 2
    assert sc_bufs * pairw + av_bufs <= 8
    W = 512 * pairw
    nc = bacc.Bacc(
        "TRN2", target_bir_lowering=False, debug=False, num_devices=N_CORES
    )
    xT = nc.dram_tensor("xT", [D, S], BF16, kind="ExternalInput")
    wqk = nc.dram_tensor("wqk", [D, FQK], BF16, kind="ExternalInput")
    wv = nc.dram_tensor("wv", [D, FV], BF16, kind="ExternalInput")
    wo = nc.dram_tensor("wo", [FV, D], BF16, kind="ExternalInput")
    bqk = nc.dram_tensor("bqk", [FQK], F32, kind="ExternalInput")
    bv = nc.dram_tensor("bv", [FV], BF16, kind="ExternalInput")
    out = nc.dram_tensor("out", [S, D], BF16, kind="ExternalOutput")

    KT = D // 128  # 8 contraction tiles over D

    with tile.TileContext(nc) as tc:
        dma = nc.sync  # HWDGE: spreads transfers over HW queues
        with (
            tc.tile_pool(name="const", bufs=1) as const,
            tc.tile_pool(name="xt", bufs=2) as xtp,
            tc.tile_pool(name="pp", bufs=2) as pp,
            tc.tile_pool(name="exp", bufs=exp_bufs) as expp,
            tc.tile_pool(name="small", bufs=4) as small,
            tc.tile_pool(name="ob", bufs=3) as obp,
            tc.tile_pool(name="ps_sc", bufs=sc_bufs, space="PSUM") as ps_sc,
            tc.tile_pool(name="ps_av", bufs=av_bufs, space="PSUM") as ps_av,
        ):
            # ---- constants ----
            wqk_sb = const.tile([128, KT, FQK], BF16)
            wv_sb = const.tile([128, KT, FV], BF16)
            wo_sb = const.tile([128, FV // 128, D], BF16)
            wqk_r = wqk.rearrange("(kt p) f -> p kt f", p=128)
            for kt in range(KT):
                dma.dma_start(
                    out=wqk_sb[:, kt:kt + 1, :], in_=wqk_r[:, kt:kt + 1, :]
                )
            dma.dma_start(
                out=wv_sb, in_=wv.rearrange("(kt p) f -> p kt f", p=128)
            )
            dma.dma_start(
                out=wo_sb, in_=wo.rearrange("(dt p) f -> p dt f", p=128)
            )
            # qk bias in partition layout [128, 4] (per-feature, f32)
            bqk_col = const.tile([128, FQK // 128], F32)
            dma.dma_start(
                out=bqk_col, in_=bqk.rearrange("(ft p) -> p ft", p=128)
            )
            bv_sb = const.tile([1, FV], BF16)
            dma.dma_start(out=bv_sb, in_=bv.rearrange("(o f) -> o f", o=1))
            ones_f32 = const.tile([1, QMB], F32)
            nc.vector.memset(ones_f32, 1.0)
            ones_row = const.tile([1, QMB], BF16)
            nc.vector.tensor_copy(ones_row, ones_f32)
            # additive causal mask for the diagonal 128x128 block:
            # trimask[k, q] = 0 if k <= q else NEG  (bf16 for fast mask MMs)
            trimask = const.tile([128, 128], F32)
            nc.gpsimd.memset(trimask, 0.0)
            nc.gpsimd.affine_select(
                out=trimask,
                in_=trimask,
                compare_op=mybir.AluOpType.is_ge,
                fill=NEG,
                base=0,
                pattern=[[1, 128]],
                channel_multiplier=-1,
            )
            trimask_r = const.tile([128, 128], BF16)
            nc.vector.tensor_copy(trimask_r, trimask)
            ident_f32 = const.tile([128, 128], F32)
            nc.gpsimd.memset(ident_f32, 0.0)
            nc.gpsimd.affine_select(
                out=ident_f32,
                in_=ident_f32,
                compare_op=mybir.AluOpType.not_equal,
                fill=1.0,
                base=0,
                pattern=[[-1, 128]],
                channel_multiplier=1,
            )
            ident_r = const.tile([128, 128], BF16)
            nc.vector.tensor_copy(ident_r, ident_f32)
            vone_f32 = const.tile([128, N_KB * HL], F32)
            nc.vector.memset(vone_f32, 1.0)

            def body(_it):
                # double-buffered across bodies: consecutive bodies use
                # alternate buffers (pp/xt pools, bufs=2), so body k+1's
                # stage A overlaps body k's B/C via plain dataflow deps.
                qkT = pp.tile([128, 4, S], BF16, tag="qkT")
                v_aug = pp.tile([128, N_KB, HL, HD + 1], BF16, tag="vaug")
                valuesT = pp.tile([128, FV // 128, S], BF16, tag="valT")
                with nc.allow_low_precision(reason="exact ones bf16"):
                    nc.vector.tensor_copy(
                        v_aug[:, :, :, HD:HD + 1],
                        vone_f32.rearrange(
                            "p (kb h o) -> p kb h o", h=HL, o=1
                        ),
                    )

                # ======== stage A: qkT and v_aug ====
                # Both x halves are DMA'd up front (xt pool is
                # double-buffered); half-1 compute items are queued as
                # PE-filler work dripped into B(qmb0/1), which only need
                # half 0.
                def load_xts(half):
                    s0 = half * (S // 2)
                    xts = []
                    for kt in range(KT):
                        xt_t = xtp.tile([128, S // 2], BF16, tag=f"xt{kt}")
                        dma.dma_start(
                            out=xt_t,
                            in_=xT[kt * 128:(kt + 1) * 128, s0:s0 + S // 2],
                        )
                        xts.append(xt_t)
                    return xts

                def make_qk_item(xts, half, ft):
                    s0 = half * (S // 2)

                    def emit():
                        ps = ps_sc.tile([128, W], F32, tag="sc")
                        for kt in range(KT):
                            for nt in range(2):
                                nc.tensor.matmul(
                                    ps[:, nt * 512:nt * 512 + 512],
                                    wqk_sb[:, kt, ft * 128:(ft + 1) * 128],
                                    xts[kt][:, nt * 512:nt * 512 + 512],
                                    start=(kt == 0),
                                    stop=(kt == KT - 1),
                                    skip_group_check=True,
                                )
                        # copy out: qkT = ps + bqk  (bias fold on DVE)
                        with nc.allow_low_precision(
                            reason="qkT stored bf16"
                        ):
                            nc.vector.tensor_scalar_add(
                                qkT[:, ft, s0:s0 + S // 2],
                                ps,
                                bqk_col[:, ft:ft + 1],
                            )
                    return emit

                def make_v_item(xts, half, stp):
                    def emit():
                        psv = ps_sc.tile([128, 512], F32, tag="sc")
                        for sub in range(2):
                            sti = stp * 2 + sub
                            c0 = sub * FV
                            for kt in range(KT):
                                nc.tensor.matmul(
                                    psv[:, c0:c0 + FV],
                                    xts[kt][:, sti * 128:(sti + 1) * 128],
                                    wv_sb[:, kt, :],
                                    start=(kt == 0),
                                    stop=False,
                                )
                            nc.tensor.matmul(
                                psv[:, c0:c0 + FV],
                                ones_row[0:1, 0:128],
                                bv_sb,
                                start=False,
                                stop=True,
                            )
                        st0 = half * 8 + stp * 2
                        nc.vector.tensor_copy(
                            v_aug[:, st0:st0 + 2, :, 0:HD],
                            psv.rearrange("s (t h c) -> s t h c", t=2, h=HL),
                        )
                    return emit

                def a_items(xts, half):
                    items = []
                    for ft in range(4):
                        items.append(make_qk_item(xts, half, ft))
                    for stp in range(4):
                        items.append(make_v_item(xts, half, stp))
                    return items

                xts0 = load_xts(0)
                xts1 = load_xts(1)
                for it in a_items(xts0, 0):
                    it()
                filler = list(a_items(xts1, 1))

                if "B" not in stages:
                    for it in filler:
                        it()
                    dma.dma_start(
                        out=out[0:128, 0:512],
                        in_=qkT[:, 0, 0:512],
                    )
                    return

                # ======== stage B+C: per query macro-block ========
                # A-half1 and C work are drip-fed into B's matmul stream
                # as PE filler (keeps the PE HAM-warm).

                def make_c_item(st):
                    def emit():
                        ob = obp.tile([128, 1024], BF16)
                        for nt in range(2):
                            ps = ps_sc.tile([128, W], F32, tag="sc")
                            for dt_ in range(FV // 128):
                                nc.tensor.matmul(
                                    ps[:, 0:512],
                                    valuesT[:, dt_, st * 128:(st + 1) * 128],
                                    wo_sb[:, dt_, nt * 512:(nt + 1) * 512],
                                    start=(dt_ == 0),
                                    stop=(dt_ == FV // 128 - 1),
                                )
                            nc.vector.tensor_copy(
                                ob[:, nt * 512:(nt + 1) * 512], ps[:, 0:512]
                            )
                        dma.dma_start(
                            out=out[st * 128:(st + 1) * 128, :], in_=ob
                        )
                    return emit

                for qmb in range(N_QMB):
                    if qmb == 2:
                        while filler:
                            filler.pop(0)()
                    q0 = qmb * QMB
                    nkb = 4 * qmb + 4
                    nblk = nkb // pairw
                    for w0 in range(0, HL, wave):
                        whs = list(range(w0, w0 + wave))
                        avs = {
                            h_: ps_av.tile([65, QMB], F32, tag="av",
                                           name=f"av{h_}")
                            for h_ in whs
                        }
                        avq = []

                        def emit_av(item):
                            h, mms = item
                            for mm in mms:
                                _, kb, col0, avw, ex_t = mm
                                nc.tensor.matmul(
                                    avs[h][0:65, col0:col0 + avw],
                                    v_aug[:, kb, h, :],
                                    ex_t,
                                    start=(kb == 0),
                                    stop=(kb == nkb - 1),
                                )

                        for blk in range(nblk):
                            kb0 = blk * pairw
                            diag = kb0 + pairw - 1 >= 4 * qmb
                            scs = {}
                            # row-packed: both heads' score MMs emitted
                            # back-to-back; lhsT base partitions 0/64 ->
                            # concurrent row-group execution on the PE.
                            for h in whs:
                                scs[h] = ps_sc.tile(
                                    [128, W], F32, tag="sc",
                                    name=f"sc{h}"
                                )
                            for sub in range(pairw):
                                kb = kb0 + sub
                                j = kb - 4 * qmb
                                col0 = 128 * j if j >= 0 else 0
                                cb = sub * 512 + col0
                                scw = 512 - col0
                                for h in whs:
                                    tk = 2 * (h // 2)
                                    pk = 64 * (h % 2)
                                    nc.tensor.matmul(
                                        scs[h][:, cb:cb + scw],
                                        qkT[pk:pk + 64, tk,
                                            kb * KB:(kb + 1) * KB],
                                        qkT[pk:pk + 64, tk + 1,
                                            q0 + col0:q0 + col0 + scw],
                                        start=True,
                                        stop=(j < 0),
                                        skip_group_check=True,
                                    )
                            if diag:
                                for h in whs:
                                    for sub in range(pairw):
                                        j = kb0 + sub - 4 * qmb
                                        if j < 0:
                                            continue
                                        cb = sub * 512 + 128 * j
                                        nc.tensor.matmul(
                                            scs[h][:, cb:cb + 128],
                                            ident_r,
                                            trimask_r,
                                            start=False,
                                            stop=True,
                                            skip_group_check=True,
                                        )
                            for h in whs:
                                sc = scs[h]
                                ex = expp.tile([128, W], BF16)
                                if diag and not fullexp:
                                    for sub in range(pairw):
                                        j = kb0 + sub - 4 * qmb
                                        col0 = 128 * j if j >= 0 else 0
                                        cb = sub * 512 + col0
                                        nc.scalar.activation(
                                            out=ex[:, cb:sub * 512 + 512],
                                            in_=sc[:, cb:sub * 512 + 512],
                                            func=(mybir
                                                  .ActivationFunctionType.Exp),
                                            scale=SCALE,
                                        )
                                else:
                                    # one full-tile exp; the below-diagonal
                                    # garbage region of ex is never read by
                                    # the av matmuls (they slice [col0:512]).
                                    nc.scalar.activation(
                                        out=ex,
                                        in_=sc,
                                        func=mybir.ActivationFunctionType.Exp,
                                        scale=SCALE,
                                    )
                                mms = []
                                for sub in range(pairw):
                                    kb = kb0 + sub
                                    j = kb - 4 * qmb
                                    col0 = 128 * j if j >= 0 else 0
                                    avw = QMB - col0
                                    mms.append((
                                        "sg", kb, col0, avw,
                                        ex[:, sub * 512 + col0:
                                            sub * 512 + col0 + avw],
                                    ))
                                avq.append((h, mms))
                            can_pop = qmb < 2 or blk > 0
                            if fill_first and filler and can_pop:
                                filler.pop(0)()
                            while len(avq) > wave * lag:
                                emit_av(avq.pop(0))
                            if not fill_first and filler and can_pop:
                                filler.pop(0)()
                        # drain + normalize: values = av[0:64] / av[64].
                        # Each head's normalize chain (DVE recip -> Pool
                        # broadcast -> DVE mul) is emitted as soon as that
                        # head's last av matmul lands, starting the DVE
                        # work earlier and freeing the av PSUM bank sooner
                        # for the next wave.
                        def normalize(h):
                            av = avs[h]
                            rec = small.tile([1, QMB], F32R, tag="rec")
                            with nc.allow_low_precision(
                                reason="softmax denom feeds bf16 matmul"
                            ):
                                nc.vector.reciprocal(rec, av[64:65, :])
                            rb = small.tile([64, QMB], F32R, tag="rb")
                            nc.gpsimd.partition_broadcast(rb, rec)
                            dt_ = h // 2
                            pr = 64 * (h % 2)
                            with nc.allow_low_precision(
                                reason="attn values stored bf16"
                            ):
                                nc.vector.tensor_mul(
                                    valuesT[pr:pr + 64, dt_, q0:q0 + QMB],
                                    av[0:64, :],
                                    rb,
                                )

                        rest = list(avq)
                        avq.clear()
                        for i, item in enumerate(rest):
                            emit_av(item)
                            h_done = item[0]
                            if not any(
                                it[0] == h_done for it in rest[i + 1:]
                            ):
                                normalize(h_done)
                    # ---- queue stage C for this qmb ----
                    if "C" not in stages:
                        continue
                    for sti in range(QMB // 128):
                        filler.append(make_c_item(qmb * 4 + sti))
                while filler:
                    filler.pop(0)()

            if repeat == 1:
                body(0)
            else:
                n_loop = repeat // unroll
                rem = repeat - n_loop * unroll
                if n_loop > 0:
                    with tc.For_i(
                        0, n_loop, 1,
                        hint_engines=(mybir.EngineType.PE,),
                        staggered_reset=staggered,
                    ) as it:
                        for _u in range(unroll):
                            body(it)
                for _u in range(rem):
                    body(0)
    nc.compile()
    return nc


def make_in_maps(x, W_qkv, b_qkv, W_out, b_out):
    """Host-side sharding: per-core input dict (fp8/bf16, W pre-scaled)."""
    x = np.asarray(x, dtype=np.float32)
    W_qkv = np.asarray(W_qkv, dtype=np.float32)
    b_qkv = np.asarray(b_qkv, dtype=np.float32)
    W_out = np.asarray(W_out, dtype=np.float32)
    in_maps = []
    xT_by_b = [
        np.ascontiguousarray(x[b_].T.astype(NP_BF16)) for b_ in range(B)
    ]
    for c in range(N_CORES):
        b_ = c // 4
        g = c % 4
        heads = [4 * g + i for i in range(HL)]
        # feature order: K(h0),K(h1),Q(h0),Q(h1),K(h2),K(h3),Q(h2),Q(h3)
        qk_cols = []
        for pair in range(2):
            h0, h1 = heads[2 * pair], heads[2 * pair + 1]
            for h_ in (h0, h1):
                base = h_ * 3 * HD + 1 * HD  # K
                qk_cols.extend(range(base, base + HD))
            for h_ in (h0, h1):
                base = h_ * 3 * HD + 0 * HD  # Q
                qk_cols.extend(range(base, base + HD))
        v_cols = []
        for h_ in heads:
            base = h_ * 3 * HD + 2 * HD  # V
            v_cols.extend(range(base, base + HD))
        qk_cols = np.array(qk_cols)
        v_cols = np.array(v_cols)
        in_maps.append({
            "xT": xT_by_b[b_],
            "wqk": np.ascontiguousarray(W_qkv[:, qk_cols].astype(NP_BF16)),
            "wv": np.ascontiguousarray(W_qkv[:, v_cols].astype(NP_BF16)),
            "wo": np.ascontiguousarray(
                W_out[g * FV:(g + 1) * FV, :].astype(NP_BF16)
            ),
            "bqk": np.ascontiguousarray(b_qkv[qk_cols].astype(np.float32)),
            "bv": np.ascontiguousarray(b_qkv[v_cols].astype(NP_BF16)),
        })
    return in_maps


_NC_CACHE = {}


def get_nc(repeat: int = 1):
    if repeat not in _NC_CACHE:
        _NC_CACHE[repeat] = build_kernel(repeat)
    return _NC_CACHE[repeat]


def kernel(x, W_qkv, b_qkv, W_out, b_out):
    in_maps = make_in_maps(x, W_qkv, b_qkv, W_out, b_out)
    nc = get_nc(1)
    res = run_bass_kernel_spmd(nc, in_maps, list(range(N_CORES)))
    b_out = np.asarray(b_out, dtype=np.float32)
    out = np.zeros((B, S, D), dtype=np.float32)
    for b_ in range(B):
        acc = np.zeros((S, D), dtype=np.float32)
        for g in range(4):
            acc += np.asarray(res.results[4 * b_ + g]["out"], dtype=np.float32)
        out[b_] = acc + b_out[None, :]
    return out



# revision 6
# speedup vs baseline: 1.2046x; 1.2046x over previous
"""Multi-head causal attention (B=2, S=2048, D=1024, H=16) on 8 trn2 cores.

Sharding: core c -> (batch b = c//4, head-group g = c%4, 4 heads each).
Data-parallel over B, tensor-parallel over heads. Each core computes a
partial output projection [S, D]; the host sums the 4 partials per batch
and adds b_out.

All matmul operands are bf16 (PSUM accumulation stays f32). Device kernel
per core:
  A) qkT[f=512, s=2048] = (x @ Wqk)^T (qk bias folded into the PSUM->SBUF
     copy as a DVE tensor_scalar_add) and v_aug[s, 4, hd+1] = x @ Wv + bv
     (ones col appended -> softmax denominators ride the av matmul).
     Startup DMAs are interleaved (wqk[kt] with xT[kt]) so the PE starts
     within ~2us instead of waiting for the full weight+x transfer; wo is
     loaded last (first needed ~40us in by stage C).
  B) per head h, per 512-wide query block qmb: causal flash attention in
     the scores-TRANSPOSED layout: sT[k,q] = K @ Q^T so that attn@V is
     lhsT=v_blk[s,hd+1], rhs=expT[k,q]. The causal mask inside the
     diagonal 128x128 block is applied by a DVE multiply of the exp tile
     with a 0/1 triangle (exact), not by PE mask matmuls. sc->exp->av
     chains are software-pipelined (av lags by `lag` links) over a
     rotating PSUM pool.
  C) out_partial[s, 1024] = values^T.T @ W_out, dt-outer loop so each
     valuesT stationary is loaded once; bf16 partials DMA'd to DRAM; host
     accumulates in f32. Tail C items copy PSUM->SBUF on the (by then
     idle) ACT engine to overlap with PE.
Fillers (stage-A half-1 and stage-C items) drip into B's matmul stream;
the qmb==2 filler dump only flushes A items (C items would head-of-line
block the in-order PE queue on the preceding wave's normalize).
"""
import math
import numpy as np
import ml_dtypes

import concourse.bass as bass
import concourse.mybir as mybir
import concourse.tile as tile
from concourse import bacc
from concourse.bass_utils import run_bass_kernel_spmd

N_CORES = 8
B, S, D = 2, 2048, 1024
H = 16                    # total heads
HL = 4                    # heads per core
HD = 64                   # head dim
FQK = 2 * HL * HD         # 512 local q+k features
FV = HL * HD              # 256 local v features
SCALE = 1.0 / math.sqrt(HD)
NEG = -1e9

QMB = 512                 # query macro-block
KB = 128                  # key block
N_QMB = S // QMB          # 4
N_KB = S // KB            # 16

F32 = mybir.dt.float32
F32R = mybir.dt.float32r
BF16 = mybir.dt.bfloat16
NP_BF16 = ml_dtypes.bfloat16


def build_kernel(repeat: int = 1, stages: str = "ABC",
                 bmode: str = "full", pairw: int = 2, wave: int = 2,
                 sc_bufs: int = 3, av_bufs: int = 2, exp_bufs: int = 8,
                 lag: int = 2, fullexp: bool = False,
                 fill_first: bool = False, unroll: int = 2,
                 staggered: bool = True, mask_dve: bool = True,
                 act_tail_copy: bool = True, dma_interleave: bool = True,
                 norm_chunks: int = 1, split_c_dma: bool = True):
    assert pairw == 2 and wave == 2
    assert sc_bufs * pairw + av_bufs <= 8
    W = 512 * pairw
    nc = bacc.Bacc(
        "TRN2", target_bir_lowering=False, debug=False, num_devices=N_CORES
    )
    xT = nc.dram_tensor("xT", [D, S], BF16, kind="ExternalInput")
    wqk = nc.dram_tensor("wqk", [D, FQK], BF16, kind="ExternalInput")
    wv = nc.dram_tensor("wv", [D, FV], BF16, kind="ExternalInput")
    wo = nc.dram_tensor("wo", [FV, D], BF16, kind="ExternalInput")
    bqk = nc.dram_tensor("bqk", [FQK], F32, kind="ExternalInput")
    bv = nc.dram_tensor("bv", [FV], BF16, kind="ExternalInput")
    out = nc.dram_tensor("out", [S, D], BF16, kind="ExternalOutput")

    KT = D // 128  # 8 contraction tiles over D

    with tile.TileContext(nc) as tc:
        dma = nc.sync  # HWDGE: SP-queue descriptor generation
        with (
            tc.tile_pool(name="const", bufs=1) as const,
            tc.tile_pool(name="xt", bufs=2) as xtp,
            tc.tile_pool(name="pp", bufs=2) as pp,
            tc.tile_pool(name="exp", bufs=exp_bufs) as expp,
            tc.tile_pool(name="small", bufs=4) as small,
            tc.tile_pool(name="ob", bufs=3) as obp,
            tc.tile_pool(name="ps_sc", bufs=sc_bufs, space="PSUM") as ps_sc,
            tc.tile_pool(name="ps_av", bufs=av_bufs, space="PSUM") as ps_av,
        ):
            # ---- const tiles (DMAs mostly deferred to the interleave) ----
            wqk_sb = const.tile([128, KT, FQK], BF16)
            wv_sb = const.tile([128, KT, FV], BF16)
            wo_sb = const.tile([128, FV // 128, D], BF16)
            bqk_col = const.tile([128, FQK // 128], F32)
            bv_sb = const.tile([1, FV], BF16)
            wqk_r = wqk.rearrange("(kt p) f -> p kt f", p=128)
            wv_r = wv.rearrange("(kt p) f -> p kt f", p=128)

            ones_f32 = const.tile([1, QMB], F32)
            nc.vector.memset(ones_f32, 1.0)
            ones_row = const.tile([1, QMB], BF16)
            nc.vector.tensor_copy(ones_row, ones_f32)
            # 0/1 causal mask for the diagonal 128x128 block:
            # trimask01[k, q] = 1 if k <= q else 0 (bf16: exact)
            tri01_f32 = const.tile([128, 128], F32)
            nc.gpsimd.memset(tri01_f32, 1.0)
            nc.gpsimd.affine_select(
                out=tri01_f32,
                in_=tri01_f32,
                compare_op=mybir.AluOpType.is_ge,
                fill=0.0,
                base=0,
                pattern=[[1, 128]],
                channel_multiplier=-1,
            )
            trimask01 = const.tile([128, 128], BF16)
            nc.vector.tensor_copy(trimask01, tri01_f32)
            if not mask_dve:
                trimask = const.tile([128, 128], F32)
                nc.gpsimd.memset(trimask, 0.0)
                nc.gpsimd.affine_select(
                    out=trimask, in_=trimask,
                    compare_op=mybir.AluOpType.is_ge, fill=NEG,
                    base=0, pattern=[[1, 128]], channel_multiplier=-1,
                )
                trimask_r = const.tile([128, 128], BF16)
                nc.vector.tensor_copy(trimask_r, trimask)
                ident_f32 = const.tile([128, 128], F32)
                nc.gpsimd.memset(ident_f32, 0.0)
                nc.gpsimd.affine_select(
                    out=ident_f32, in_=ident_f32,
                    compare_op=mybir.AluOpType.not_equal, fill=1.0,
                    base=0, pattern=[[-1, 128]], channel_multiplier=1,
                )
                ident_r = const.tile([128, 128], BF16)
                nc.vector.tensor_copy(ident_r, ident_f32)
            vone_f32 = const.tile([128, N_KB * HL], F32)
            nc.vector.memset(vone_f32, 1.0)

            def load_xts(half, interleave_weights=False):
                """DMA one x half per kt; optionally interleave the weight
                slices so the first qk MMs can start ~2us in."""
                s0 = half * (S // 2)
                xts = []
                for kt in range(KT):
                    if interleave_weights:
                        dma.dma_start(
                            out=wqk_sb[:, kt:kt + 1, :],
                            in_=wqk_r[:, kt:kt + 1, :],
                        )
                    xt_t = xtp.tile([128, S // 2], BF16, tag=f"xt{kt}")
                    dma.dma_start(
                        out=xt_t,
                        in_=xT[kt * 128:(kt + 1) * 128, s0:s0 + S // 2],
                    )
                    xts.append(xt_t)
                    if interleave_weights and kt == 1:
                        dma.dma_start(
                            out=bqk_col,
                            in_=bqk.rearrange("(ft p) -> p ft", p=128),
                        )
                        dma.dma_start(
                            out=bv_sb, in_=bv.rearrange("(o f) -> o f", o=1)
                        )
                    if interleave_weights and kt in (3, 5):
                        k0 = 4 * ((kt - 3) // 2)
                        dma.dma_start(
                            out=wv_sb[:, k0:k0 + 4, :],
                            in_=wv_r[:, k0:k0 + 4, :],
                        )
                return xts

            def load_const_weights(skip_interleaved=False):
                if not skip_interleaved:
                    for kt in range(KT):
                        dma.dma_start(
                            out=wqk_sb[:, kt:kt + 1, :],
                            in_=wqk_r[:, kt:kt + 1, :],
                        )
                    dma.dma_start(out=wv_sb, in_=wv_r)
                    dma.dma_start(
                        out=bqk_col,
                        in_=bqk.rearrange("(ft p) -> p ft", p=128),
                    )
                    dma.dma_start(
                        out=bv_sb, in_=bv.rearrange("(o f) -> o f", o=1)
                    )
                dma.dma_start(
                    out=wo_sb, in_=wo.rearrange("(dt p) f -> p dt f", p=128)
                )

            def body(_it, xts0=None, xts1=None):
                # double-buffered across bodies: consecutive bodies use
                # alternate buffers (pp/xt pools, bufs=2), so body k+1's
                # stage A overlaps body k's B/C via plain dataflow deps.
                qkT = pp.tile([128, 4, S], BF16, tag="qkT")
                v_aug = pp.tile([128, N_KB, HL, HD + 1], BF16, tag="vaug")
                valuesT = pp.tile([128, FV // 128, S], BF16, tag="valT")
                with nc.allow_low_precision(reason="exact ones bf16"):
                    nc.vector.tensor_copy(
                        v_aug[:, :, :, HD:HD + 1],
                        vone_f32.rearrange(
                            "p (kb h o) -> p kb h o", h=HL, o=1
                        ),
                    )

                # ======== stage A: qkT and v_aug ====
                def make_qk_item(xts, half, ft):
                    s0 = half * (S // 2)

                    def emit():
                        ps = ps_sc.tile([128, W], F32, tag="sc")
                        for kt in range(KT):
                            for nt in range(2):
                                nc.tensor.matmul(
                                    ps[:, nt * 512:nt * 512 + 512],
                                    wqk_sb[:, kt, ft * 128:(ft + 1) * 128],
                                    xts[kt][:, nt * 512:nt * 512 + 512],
                                    start=(kt == 0),
                                    stop=(kt == KT - 1),
                                    skip_group_check=True,
                                )
                        # copy out: qkT = ps + bqk  (bias fold on DVE)
                        with nc.allow_low_precision(
                            reason="qkT stored bf16"
                        ):
                            nc.vector.tensor_scalar_add(
                                qkT[:, ft, s0:s0 + S // 2],
                                ps,
                                bqk_col[:, ft:ft + 1],
                            )
                    return emit

                def make_v_item(xts, half, stp):
                    def emit():
                        psv = ps_sc.tile([128, 512], F32, tag="sc")
                        for sub in range(2):
                            sti = stp * 2 + sub
                            c0 = sub * FV
                            for kt in range(KT):
                                nc.tensor.matmul(
                                    psv[:, c0:c0 + FV],
                                    xts[kt][:, sti * 128:(sti + 1) * 128],
                                    wv_sb[:, kt, :],
                                    start=(kt == 0),
                                    stop=False,
                                )
                            nc.tensor.matmul(
                                psv[:, c0:c0 + FV],
                                ones_row[0:1, 0:128],
                                bv_sb,
                                start=False,
                                stop=True,
                            )
                        st0 = half * 8 + stp * 2
                        nc.vector.tensor_copy(
                            v_aug[:, st0:st0 + 2, :, 0:HD],
                            psv.rearrange("s (t h c) -> s t h c", t=2, h=HL),
                        )
                    return emit

                def a_items(xts, half):
                    items = []
                    for ft in range(4):
                        items.append(make_qk_item(xts, half, ft))
                    for stp in range(4):
                        items.append(make_v_item(xts, half, stp))
                    return items

                if xts0 is None:
                    xts0 = load_xts(0)
                if xts1 is None:
                    xts1 = load_xts(1)
                for it in a_items(xts0, 0):
                    it()
                # filler: list of (kind, emit) -- 'A' items must flush by
                # qmb==2 (qkT half-1 needed); 'C' items are deferred pops.
                filler = [("A", it) for it in a_items(xts1, 1)]

                def pop_filler():
                    for i, (kind, _fn) in enumerate(filler):
                        if kind == "A":
                            filler.pop(i)[1]()
                            return
                    filler.pop(0)[1]()

                if "B" not in stages:
                    while filler:
                        filler.pop(0)[1]()
                    dma.dma_start(
                        out=out[0:128, 0:512],
                        in_=qkT[:, 0, 0:512],
                    )
                    return

                # ======== stage B+C: per query macro-block ========
                def make_c_item(st, use_act=False):
                    def emit():
                        ob = obp.tile([128, 1024], BF16)
                        ps = ps_sc.tile([128, W], F32, tag="sc")
                        for dt_ in range(FV // 128):
                            for nt in range(2):
                                nc.tensor.matmul(
                                    ps[:, nt * 512:(nt + 1) * 512],
                                    valuesT[:, dt_, st * 128:(st + 1) * 128],
                                    wo_sb[:, dt_, nt * 512:(nt + 1) * 512],
                                    start=(dt_ == 0),
                                    stop=(dt_ == FV // 128 - 1),
                                    skip_group_check=True,
                                )
                        for nt in range(2):
                            # tail items split the two copies across DVE
                            # (nt0) and the by-then idle ACT (nt1) so they
                            # run in parallel
                            if use_act and nt == 1:
                                with nc.allow_low_precision(
                                    reason="out partial stored bf16"
                                ):
                                    nc.scalar.activation(
                                        out=ob[:, nt * 512:(nt + 1) * 512],
                                        in_=ps[:, nt * 512:(nt + 1) * 512],
                                        func=(mybir
                                              .ActivationFunctionType.Copy),
                                    )
                            else:
                                nc.vector.tensor_copy(
                                    ob[:, nt * 512:(nt + 1) * 512],
                                    ps[:, nt * 512:(nt + 1) * 512],
                                )
                            if split_c_dma:
                                dma.dma_start(
                                    out=out[st * 128:(st + 1) * 128,
                                            nt * 512:(nt + 1) * 512],
                                    in_=ob[:, nt * 512:(nt + 1) * 512],
                                )
                        if not split_c_dma:
                            dma.dma_start(
                                out=out[st * 128:(st + 1) * 128, :], in_=ob
                            )
                    return emit

                for qmb in range(N_QMB):
                    if qmb == 2:
                        # flush remaining A items (qkT half-1 needed now);
                        # C items stay queued -- popping them here would
                        # head-of-line block the PE on the normalize of
                        # the wave that just ended.
                        for i in range(len(filler) - 1, -1, -1):
                            if filler[i][0] == "A":
                                filler.pop(i)[1]()
                    q0 = qmb * QMB
                    nkb = 4 * qmb + 4
                    nblk = nkb // pairw
                    for w0 in range(0, HL, wave):
                        whs = list(range(w0, w0 + wave))
                        avs = {
                            h_: ps_av.tile([65, QMB], F32, tag="av",
                                           name=f"av{h_}")
                            for h_ in whs
                        }
                        avq = []

                        def emit_av(item):
                            h, mms = item
                            for mm in mms:
                                _, kb, col0, avw, ex_t = mm
                                nc.tensor.matmul(
                                    avs[h][0:65, col0:col0 + avw],
                                    v_aug[:, kb, h, :],
                                    ex_t,
                                    start=(kb == 0),
                                    stop=(kb == nkb - 1),
                                )

                        for blk in range(nblk):
                            kb0 = blk * pairw
                            diag = kb0 + pairw - 1 >= 4 * qmb
                            scs = {}
                            # row-packed: both heads' score MMs emitted
                            # back-to-back; lhsT base partitions 0/64 ->
                            # concurrent row-group execution on the PE.
                            for h in whs:
                                scs[h] = ps_sc.tile(
                                    [128, W], F32, tag="sc",
                                    name=f"sc{h}"
                                )
                            for sub in range(pairw):
                                kb = kb0 + sub
                                j = kb - 4 * qmb
                                col0 = 128 * j if j >= 0 else 0
                                cb = sub * 512 + col0
                                scw = 512 - col0
                                for h in whs:
                                    tk = 2 * (h // 2)
                                    pk = 64 * (h % 2)
                                    nc.tensor.matmul(
                                        scs[h][:, cb:cb + scw],
                                        qkT[pk:pk + 64, tk,
                                            kb * KB:(kb + 1) * KB],
                                        qkT[pk:pk + 64, tk + 1,
                                            q0 + col0:q0 + col0 + scw],
                                        start=True,
                                        stop=(True if mask_dve else (j < 0)),
                                        skip_group_check=True,
                                    )
                            if diag and not mask_dve:
                                for h in whs:
                                    for sub in range(pairw):
                                        j = kb0 + sub - 4 * qmb
                                        if j < 0:
                                            continue
                                        cb = sub * 512 + 128 * j
                                        nc.tensor.matmul(
                                            scs[h][:, cb:cb + 128],
                                            ident_r,
                                            trimask_r,
                                            start=False,
                                            stop=True,
                                            skip_group_check=True,
                                        )
                            for h in whs:
                                sc = scs[h]
                                ex = expp.tile([128, W], BF16)
                                if diag and not fullexp:
                                    for sub in range(pairw):
                                        j = kb0 + sub - 4 * qmb
                                        col0 = 128 * j if j >= 0 else 0
                                        cb = sub * 512 + col0
                                        nc.scalar.activation(
                                            out=ex[:, cb:sub * 512 + 512],
                                            in_=sc[:, cb:sub * 512 + 512],
                                            func=(mybir
                                                  .ActivationFunctionType.Exp),
                                            scale=SCALE,
                                        )
                                else:
                                    nc.scalar.activation(
                                        out=ex,
                                        in_=sc,
                                        func=mybir.ActivationFunctionType.Exp,
                                        scale=SCALE,
                                    )
                                if diag and mask_dve:
                                    # exact 0/1 triangle multiply on the
                                    # diagonal block (replaces PE mask MMs)
                                    for sub in range(pairw):
                                        j = kb0 + sub - 4 * qmb
                                        if j < 0:
                                            continue
                                        cb = sub * 512 + 128 * j
                                        with nc.allow_low_precision(
                                            reason="exact 0/1 mask"
                                        ):
                                            nc.vector.tensor_mul(
                                                ex[:, cb:cb + 128],
                                                ex[:, cb:cb + 128],
                                                trimask01,
                                            )
                                mms = []
                                for sub in range(pairw):
                                    kb = kb0 + sub
                                    j = kb - 4 * qmb
                                    col0 = 128 * j if j >= 0 else 0
                                    avw = QMB - col0
                                    mms.append((
                                        "sg", kb, col0, avw,
                                        ex[:, sub * 512 + col0:
                                            sub * 512 + col0 + avw],
                                    ))
                                avq.append((h, mms))
                            can_pop = qmb < 2 or blk > 0
                            if fill_first and filler and can_pop:
                                pop_filler()
                            while len(avq) > wave * lag:
                                emit_av(avq.pop(0))
                            if not fill_first and filler and can_pop:
                                pop_filler()
                        # drain + normalize: values = av[0:64] / av[64].
                        # Chunked over q so the recip->broadcast->mul links
                        # pipeline and the chain latency roughly halves.
                        def normalize(h):
                            av = avs[h]
                            dt_ = h // 2
                            pr = 64 * (h % 2)
                            cw = QMB // norm_chunks
                            for ci in range(norm_chunks):
                                c0 = ci * cw
                                rec = small.tile([1, QMB], F32R, tag="rec")
                                with nc.allow_low_precision(
                                    reason="softmax denom feeds bf16 matmul"
                                ):
                                    nc.vector.reciprocal(
                                        rec[:, 0:cw], av[64:65, c0:c0 + cw]
                                    )
                                rb = small.tile([64, QMB], F32R, tag="rb")
                                nc.gpsimd.partition_broadcast(
                                    rb[:, 0:cw], rec[:, 0:cw]
                                )
                                with nc.allow_low_precision(
                                    reason="attn values stored bf16"
                                ):
                                    nc.vector.tensor_mul(
                                        valuesT[pr:pr + 64, dt_,
                                                q0 + c0:q0 + c0 + cw],
                                        av[0:64, c0:c0 + cw],
                                        rb[:, 0:cw],
                                    )

                        rest = list(avq)
                        avq.clear()
                        for i, item in enumerate(rest):
                            emit_av(item)
                            h_done = item[0]
                            if not any(
                                it[0] == h_done for it in rest[i + 1:]
                            ):
                                normalize(h_done)
                    # ---- queue stage C for this qmb ----
                    if "C" not in stages:
                        continue
                    use_act = act_tail_copy and qmb == N_QMB - 1
                    for sti in range(QMB // 128):
                        filler.append(
                            ("C", make_c_item(qmb * 4 + sti, use_act))
                        )
                while filler:
                    filler.pop(0)[1]()

            if repeat == 1:
                if dma_interleave:
                    xts0 = load_xts(0, interleave_weights=True)
                    xts1 = load_xts(1)
                    load_const_weights(skip_interleaved=True)
                    body(0, xts0, xts1)
                else:
                    load_const_weights()
                    body(0)
            else:
                load_const_weights()
                n_loop = repeat // unroll
                rem = repeat - n_loop * unroll
                if n_loop > 0:
                    with tc.For_i(
                        0, n_loop, 1,
                        hint_engines=(mybir.EngineType.PE,),
                        staggered_reset=staggered,
                    ) as it:
                        for _u in range(unroll):
                            body(it)
                for _u in range(rem):
                    body(0)
    nc.compile()
    return nc


def make_in_maps(x, W_qkv, b_qkv, W_out, b_out):
    """Host-side sharding: per-core input dict (bf16 weights/activations)."""
    x = np.asarray(x, dtype=np.float32)
    W_qkv = np.asarray(W_qkv, dtype=np.float32)
    b_qkv = np.asarray(b_qkv, dtype=np.float32)
    W_out = np.asarray(W_out, dtype=np.float32)
    in_maps = []
    xT_by_b = [
        np.ascontiguousarray(x[b_].T.astype(NP_BF16)) for b_ in range(B)
    ]
    for c in range(N_CORES):
        b_ = c // 4
        g = c % 4
        heads = [4 * g + i for i in range(HL)]
        # feature order: K(h0),K(h1),Q(h0),Q(h1),K(h2),K(h3),Q(h2),Q(h3)
        qk_cols = []
        for pair in range(2):
            h0, h1 = heads[2 * pair], heads[2 * pair + 1]
            for h_ in (h0, h1):
                base = h_ * 3 * HD + 1 * HD  # K
                qk_cols.extend(range(base, base + HD))
            for h_ in (h0, h1):
                base = h_ * 3 * HD + 0 * HD  # Q
                qk_cols.extend(range(base, base + HD))
        v_cols = []
        for h_ in heads:
            base = h_ * 3 * HD + 2 * HD  # V
            v_cols.extend(range(base, base + HD))
        qk_cols = np.array(qk_cols)
        v_cols = np.array(v_cols)
        in_maps.append({
            "xT": xT_by_b[b_],
            "wqk": np.ascontiguousarray(W_qkv[:, qk_cols].astype(NP_BF16)),
            "wv": np.ascontiguousarray(W_qkv[:, v_cols].astype(NP_BF16)),
            "wo": np.ascontiguousarray(
                W_out[g * FV:(g + 1) * FV, :].astype(NP_BF16)
            ),
            "bqk": np.ascontiguousarray(b_qkv[qk_cols].astype(np.float32)),
            "bv": np.ascontiguousarray(b_qkv[v_cols].astype(NP_BF16)),
        })
    return in_maps


_NC_CACHE = {}


def get_nc(repeat: int = 1, **kw):
    key = (repeat, tuple(sorted(kw.items())))
    if key not in _NC_CACHE:
        _NC_CACHE[key] = build_kernel(repeat, **kw)
    return _NC_CACHE[key]


def kernel(x, W_qkv, b_qkv, W_out, b_out):
    in_maps = make_in_maps(x, W_qkv, b_qkv, W_out, b_out)
    nc = get_nc(1)
    res = run_bass_kernel_spmd(nc, in_maps, list(range(N_CORES)))
    b_out = np.asarray(b_out, dtype=np.float32)
    out = np.zeros((B, S, D), dtype=np.float32)
    for b_ in range(B):
        acc = np.zeros((S, D), dtype=np.float32)
        for g in range(4):
            acc += np.asarray(res.results[4 * b_ + g]["out"], dtype=np.float32)
        out[b_] = acc + b_out[None, :]
    return out


# revision 15
# speedup vs baseline: 1.2337x; 1.0242x over previous
"""Multi-head causal attention (B=2, S=2048, D=1024, H=16) on 8 trn2 cores.

Sharding: core c -> (batch b = c//4, head-group g = c%4, 4 heads each).
Data-parallel over B, tensor-parallel over heads. Each core computes a
partial output projection [S, D]; the host sums the 4 partials per batch
and adds b_out.

All matmul operands are bf16 (PSUM accumulation stays f32). Device kernel
per core:
  A) qkT[f=512, s=2048] = (x @ Wqk)^T (qk bias folded into the PSUM->SBUF
     copy as a DVE tensor_scalar_add) and v_aug[s, 4, hd+1] = x @ Wv + bv
     (ones col appended -> softmax denominators ride the av matmul).
     Startup DMAs are interleaved (wqk[kt] with xT[kt]) so the PE starts
     within ~2us instead of waiting for the full weight+x transfer; wo is
     loaded last (first needed ~40us in by stage C).
  B) per head h, per 512-wide query block qmb: causal flash attention in
     the scores-TRANSPOSED layout: sT[k,q] = K @ Q^T so that attn@V is
     lhsT=v_blk[s,hd+1], rhs=expT[k,q]. The causal mask inside the
     diagonal 128x128 block is applied by a DVE multiply of the exp tile
     with a 0/1 triangle (exact), not by PE mask matmuls. sc->exp->av
     chains are software-pipelined (av lags by `lag` links) over a
     rotating PSUM pool.
  C) out_partial[s, 1024] = values^T.T @ W_out, dt-outer loop so each
     valuesT stationary is loaded once; bf16 partials DMA'd to DRAM; host
     accumulates in f32. Tail C items copy PSUM->SBUF on the (by then
     idle) ACT engine to overlap with PE.
Fillers (stage-A half-1 and stage-C items) drip into B's matmul stream;
the qmb==2 filler dump only flushes A items (C items would head-of-line
block the in-order PE queue on the preceding wave's normalize).
"""
import math
import numpy as np
import ml_dtypes

import concourse.bass as bass
import concourse.mybir as mybir
import concourse.tile as tile
from concourse import bacc
from concourse.bass_utils import run_bass_kernel_spmd

N_CORES = 8
B, S, D = 2, 2048, 1024
H = 16                    # total heads
HL = 4                    # heads per core
HD = 64                   # head dim
FQK = 2 * HL * HD         # 512 local q+k features
FV = HL * HD              # 256 local v features
SCALE = 1.0 / math.sqrt(HD)
NEG = -1e9

QMB = 512                 # query macro-block
KB = 128                  # key block
N_QMB = S // QMB          # 4
N_KB = S // KB            # 16

F32 = mybir.dt.float32
F32R = mybir.dt.float32r
BF16 = mybir.dt.bfloat16
NP_BF16 = ml_dtypes.bfloat16


def build_kernel(repeat: int = 1, stages: str = "ABC",
                 bmode: str = "full", pairw: int = 2, wave: int = 2,
                 sc_bufs: int = 3, av_bufs: int = 2, exp_bufs: int = 8,
                 lag: int = 2, fullexp: bool = False,
                 fill_first: bool = False, unroll: int = 2,
                 staggered: bool = True, mask_dve: bool = True,
                 act_tail_copy: bool = True, dma_interleave: bool = True,
                 norm_chunks: int = 1, split_c_dma: bool = True):
    assert pairw == 2 and wave == 2
    assert sc_bufs * pairw + av_bufs <= 8
    W = 512 * pairw
    nc = bacc.Bacc(
        "TRN2", target_bir_lowering=False, debug=False, num_devices=N_CORES
    )
    xT = nc.dram_tensor("xT", [D, S], BF16, kind="ExternalInput")
    wqk = nc.dram_tensor("wqk", [D, FQK], BF16, kind="ExternalInput")
    wv = nc.dram_tensor("wv", [D, FV], BF16, kind="ExternalInput")
    wo = nc.dram_tensor("wo", [FV, D], BF16, kind="ExternalInput")
    bqk = nc.dram_tensor("bqk", [FQK], F32, kind="ExternalInput")
    bv = nc.dram_tensor("bv", [FV], BF16, kind="ExternalInput")
    out = nc.dram_tensor("out", [S, D], BF16, kind="ExternalOutput")

    KT = D // 128  # 8 contraction tiles over D

    with tile.TileContext(nc) as tc:
        dma = nc.sync  # HWDGE: SP-queue descriptor generation
        with (
            tc.tile_pool(name="const", bufs=1) as const,
            tc.tile_pool(name="xt", bufs=2) as xtp,
            tc.tile_pool(name="pp", bufs=2) as pp,
            tc.tile_pool(name="exp", bufs=exp_bufs) as expp,
            tc.tile_pool(name="small", bufs=4) as small,
            tc.tile_pool(name="ob", bufs=4) as obp,
            tc.tile_pool(name="ps_sc", bufs=sc_bufs, space="PSUM") as ps_sc,
            tc.tile_pool(name="ps_av", bufs=av_bufs, space="PSUM") as ps_av,
        ):
            # ---- const tiles (DMAs mostly deferred to the interleave) ----
            wqk_sb = const.tile([128, KT, FQK], BF16)
            wv_sb = const.tile([128, KT, FV], BF16)
            wo_sb = const.tile([128, FV // 128, D], BF16)
            bqk_col = const.tile([128, FQK // 128], F32)
            bv_sb = const.tile([1, FV], BF16)
            wqk_r = wqk.rearrange("(kt p) f -> p kt f", p=128)
            wv_r = wv.rearrange("(kt p) f -> p kt f", p=128)

            ones_f32 = const.tile([1, QMB], F32)
            nc.vector.memset(ones_f32, 1.0)
            ones_row = const.tile([1, QMB], BF16)
            nc.vector.tensor_copy(ones_row, ones_f32)
            # 0/1 causal mask for the diagonal 128x128 block:
            # trimask01[k, q] = 1 if k <= q else 0 (bf16: exact)
            tri01_f32 = const.tile([128, 128], F32)
            nc.gpsimd.memset(tri01_f32, 1.0)
            nc.gpsimd.affine_select(
                out=tri01_f32,
                in_=tri01_f32,
                compare_op=mybir.AluOpType.is_ge,
                fill=0.0,
                base=0,
                pattern=[[1, 128]],
                channel_multiplier=-1,
            )
            trimask01 = const.tile([128, 128], BF16)
            nc.vector.tensor_copy(trimask01, tri01_f32)
            if not mask_dve:
                trimask = const.tile([128, 128], F32)
                nc.gpsimd.memset(trimask, 0.0)
                nc.gpsimd.affine_select(
                    out=trimask, in_=trimask,
                    compare_op=mybir.AluOpType.is_ge, fill=NEG,
                    base=0, pattern=[[1, 128]], channel_multiplier=-1,
                )
                trimask_r = const.tile([128, 128], BF16)
                nc.vector.tensor_copy(trimask_r, trimask)
                ident_f32 = const.tile([128, 128], F32)
                nc.gpsimd.memset(ident_f32, 0.0)
                nc.gpsimd.affine_select(
                    out=ident_f32, in_=ident_f32,
                    compare_op=mybir.AluOpType.not_equal, fill=1.0,
                    base=0, pattern=[[-1, 128]], channel_multiplier=1,
                )
                ident_r = const.tile([128, 128], BF16)
                nc.vector.tensor_copy(ident_r, ident_f32)
            vone_f32 = const.tile([128, N_KB * HL], F32)
            nc.vector.memset(vone_f32, 1.0)
            # bv broadcast to all partitions (and both sub-tiles) so the
            # v bias rides the psv->v_aug DVE copy instead of PE matmuls.
            # NOTE: must be emitted AFTER the bv_sb DMA (tile deps follow
            # emission order).
            bv2 = const.tile([128, 2, HL, HD], BF16)

            def emit_bv2():
                for t_ in range(2):
                    nc.gpsimd.partition_broadcast(
                        bv2[:, t_, :, :],
                        bv_sb.rearrange("o (h c) -> o h c", h=HL),
                    )

            def load_xts(half, interleave_weights=False):
                """DMA one x half per kt; optionally interleave the weight
                slices so the first qk MMs can start ~2us in."""
                s0 = half * (S // 2)
                xts = []
                for kt in range(KT):
                    if interleave_weights:
                        dma.dma_start(
                            out=wqk_sb[:, kt:kt + 1, :],
                            in_=wqk_r[:, kt:kt + 1, :],
                        )
                    xt_t = xtp.tile([128, S // 2], BF16, tag=f"xt{kt}")
                    dma.dma_start(
                        out=xt_t,
                        in_=xT[kt * 128:(kt + 1) * 128, s0:s0 + S // 2],
                    )
                    xts.append(xt_t)
                    if interleave_weights and kt == 1:
                        dma.dma_start(
                            out=bqk_col,
                            in_=bqk.rearrange("(ft p) -> p ft", p=128),
                        )
                        dma.dma_start(
                            out=bv_sb, in_=bv.rearrange("(o f) -> o f", o=1)
                        )
                    if interleave_weights and kt in (3, 5):
                        k0 = 4 * ((kt - 3) // 2)
                        dma.dma_start(
                            out=wv_sb[:, k0:k0 + 4, :],
                            in_=wv_r[:, k0:k0 + 4, :],
                        )
                return xts

            def load_const_weights(skip_interleaved=False):
                if not skip_interleaved:
                    for kt in range(KT):
                        dma.dma_start(
                            out=wqk_sb[:, kt:kt + 1, :],
                            in_=wqk_r[:, kt:kt + 1, :],
                        )
                    dma.dma_start(out=wv_sb, in_=wv_r)
                    dma.dma_start(
                        out=bqk_col,
                        in_=bqk.rearrange("(ft p) -> p ft", p=128),
                    )
                    dma.dma_start(
                        out=bv_sb, in_=bv.rearrange("(o f) -> o f", o=1)
                    )
                dma.dma_start(
                    out=wo_sb, in_=wo.rearrange("(dt p) f -> p dt f", p=128)
                )

            def body(_it, xts0=None, xts1=None):
                # double-buffered across bodies: consecutive bodies use
                # alternate buffers (pp/xt pools, bufs=2), so body k+1's
                # stage A overlaps body k's B/C via plain dataflow deps.
                qkT = pp.tile([128, 4, S], BF16, tag="qkT")
                v_aug = pp.tile([128, N_KB, HL, HD + 1], BF16, tag="vaug")
                valuesT = pp.tile([128, FV // 128, S], BF16, tag="valT")
                with nc.allow_low_precision(reason="exact ones bf16"):
                    nc.vector.tensor_copy(
                        v_aug[:, :, :, HD:HD + 1],
                        vone_f32.rearrange(
                            "p (kb h o) -> p kb h o", h=HL, o=1
                        ),
                    )

                # ======== stage A: qkT and v_aug ====
                def make_qk_item(xts, half, ft):
                    s0 = half * (S // 2)

                    def emit():
                        ps = ps_sc.tile([128, W], F32, tag="sc")
                        for kt in range(KT):
                            for nt in range(2):
                                nc.tensor.matmul(
                                    ps[:, nt * 512:nt * 512 + 512],
                                    wqk_sb[:, kt, ft * 128:(ft + 1) * 128],
                                    xts[kt][:, nt * 512:nt * 512 + 512],
                                    start=(kt == 0),
                                    stop=(kt == KT - 1),
                                    skip_group_check=True,
                                )
                        # copy out: qkT = ps + bqk  (bias fold on DVE)
                        with nc.allow_low_precision(
                            reason="qkT stored bf16"
                        ):
                            nc.vector.tensor_scalar_add(
                                qkT[:, ft, s0:s0 + S // 2],
                                ps,
                                bqk_col[:, ft:ft + 1],
                            )
                    return emit

                def make_v_item(xts, half, stp):
                    def emit():
                        psv = ps_sc.tile([128, 512], F32, tag="sc")
                        for sub in range(2):
                            sti = stp * 2 + sub
                            c0 = sub * FV
                            for kt in range(KT):
                                nc.tensor.matmul(
                                    psv[:, c0:c0 + FV],
                                    xts[kt][:, sti * 128:(sti + 1) * 128],
                                    wv_sb[:, kt, :],
                                    start=(kt == 0),
                                    stop=(kt == KT - 1),
                                )
                        st0 = half * 8 + stp * 2
                        # bias folded into the copy (bv2 pre-broadcast)
                        with nc.allow_low_precision(
                            reason="v values stored bf16"
                        ):
                            nc.vector.tensor_add(
                                v_aug[:, st0:st0 + 2, :, 0:HD],
                                psv.rearrange(
                                    "s (t h c) -> s t h c", t=2, h=HL
                                ),
                                bv2,
                            )
                    return emit

                def a_items(xts, half):
                    items = []
                    for ft in range(4):
                        items.append(make_qk_item(xts, half, ft))
                    for stp in range(4):
                        items.append(make_v_item(xts, half, stp))
                    return items

                if xts0 is None:
                    xts0 = load_xts(0)
                if xts1 is None:
                    xts1 = load_xts(1)
                for it in a_items(xts0, 0):
                    it()
                # filler: list of (kind, emit) -- 'A' items must flush by
                # qmb==2 (qkT half-1 needed); 'C' items are deferred pops.
                filler = [("A", it) for it in a_items(xts1, 1)]

                def pop_filler():
                    for i, (kind, _fn) in enumerate(filler):
                        if kind == "A":
                            filler.pop(i)[1]()
                            return
                    filler.pop(0)[1]()

                if "B" not in stages:
                    while filler:
                        filler.pop(0)[1]()
                    dma.dma_start(
                        out=out[0:128, 0:512],
                        in_=qkT[:, 0, 0:512],
                    )
                    return

                # ======== stage B+C: per query macro-block ========
                def make_c_item(st, use_act=False):
                    def emit():
                        ob = obp.tile([128, 1024], BF16)
                        ps = ps_sc.tile([128, W], F32, tag="sc")
                        for dt_ in range(FV // 128):
                            for nt in range(2):
                                nc.tensor.matmul(
                                    ps[:, nt * 512:(nt + 1) * 512],
                                    valuesT[:, dt_, st * 128:(st + 1) * 128],
                                    wo_sb[:, dt_, nt * 512:(nt + 1) * 512],
                                    start=(dt_ == 0),
                                    stop=(dt_ == FV // 128 - 1),
                                    skip_group_check=True,
                                )
                        for nt in range(2):
                            # tail items split the two copies across DVE
                            # (nt0) and the by-then idle ACT (nt1) so they
                            # run in parallel
                            if use_act and nt == 1:
                                with nc.allow_low_precision(
                                    reason="out partial stored bf16"
                                ):
                                    nc.scalar.activation(
                                        out=ob[:, nt * 512:(nt + 1) * 512],
                                        in_=ps[:, nt * 512:(nt + 1) * 512],
                                        func=(mybir
                                              .ActivationFunctionType.Copy),
                                    )
                            else:
                                nc.vector.tensor_copy(
                                    ob[:, nt * 512:(nt + 1) * 512],
                                    ps[:, nt * 512:(nt + 1) * 512],
                                )
                            # tail items use one combined DMA: the 625ns
                            # HWDGE descriptor-gen serializes and is the
                            # tail bottleneck, not transfer overlap
                            if split_c_dma and not use_act:
                                dma.dma_start(
                                    out=out[st * 128:(st + 1) * 128,
                                            nt * 512:(nt + 1) * 512],
                                    in_=ob[:, nt * 512:(nt + 1) * 512],
                                )
                        if not (split_c_dma and not use_act):
                            dma.dma_start(
                                out=out[st * 128:(st + 1) * 128, :], in_=ob
                            )
                    return emit

                for qmb in range(N_QMB):
                    if qmb == 2:
                        # flush remaining A items (qkT half-1 needed now);
                        # C items stay queued -- popping them here would
                        # head-of-line block the PE on the normalize of
                        # the wave that just ended.
                        for i in range(len(filler) - 1, -1, -1):
                            if filler[i][0] == "A":
                                filler.pop(i)[1]()
                    q0 = qmb * QMB
                    nkb = 4 * qmb + 4
                    nblk = nkb // pairw
                    for w0 in range(0, HL, wave):
                        whs = list(range(w0, w0 + wave))
                        avs = {
                            h_: ps_av.tile([65, QMB], F32, tag="av",
                                           name=f"av{h_}")
                            for h_ in whs
                        }
                        avq = []

                        def emit_av(item):
                            h, mms = item
                            for mm in mms:
                                _, kb, col0, avw, ex_t = mm
                                nc.tensor.matmul(
                                    avs[h][0:65, col0:col0 + avw],
                                    v_aug[:, kb, h, :],
                                    ex_t,
                                    start=(kb == 0),
                                    stop=(kb == nkb - 1),
                                )

                        for blk in range(nblk):
                            kb0 = blk * pairw
                            diag = kb0 + pairw - 1 >= 4 * qmb
                            scs = {}
                            # row-packed: both heads' score MMs emitted
                            # back-to-back; lhsT base partitions 0/64 ->
                            # concurrent row-group execution on the PE.
                            for h in whs:
                                scs[h] = ps_sc.tile(
                                    [128, W], F32, tag="sc",
                                    name=f"sc{h}"
                                )
                            for sub in range(pairw):
                                kb = kb0 + sub
                                j = kb - 4 * qmb
                                col0 = 128 * j if j >= 0 else 0
                                cb = sub * 512 + col0
                                scw = 512 - col0
                                for h in whs:
                                    tk = 2 * (h // 2)
                                    pk = 64 * (h % 2)
                                    nc.tensor.matmul(
                                        scs[h][:, cb:cb + scw],
                                        qkT[pk:pk + 64, tk,
                                            kb * KB:(kb + 1) * KB],
                                        qkT[pk:pk + 64, tk + 1,
                                            q0 + col0:q0 + col0 + scw],
                                        start=True,
                                        stop=(True if mask_dve else (j < 0)),
                                        skip_group_check=True,
                                    )
                            if diag and not mask_dve:
                                for h in whs:
                                    for sub in range(pairw):
                                        j = kb0 + sub - 4 * qmb
                                        if j < 0:
                                            continue
                                        cb = sub * 512 + 128 * j
                                        nc.tensor.matmul(
                                            scs[h][:, cb:cb + 128],
                                            ident_r,
                                            trimask_r,
                                            start=False,
                                            stop=True,
                                            skip_group_check=True,
                                        )
                            for h in whs:
                                sc = scs[h]
                                ex = expp.tile([128, W], BF16)
                                if diag and not fullexp:
                                    for sub in range(pairw):
                                        j = kb0 + sub - 4 * qmb
                                        col0 = 128 * j if j >= 0 else 0
                                        cb = sub * 512 + col0
                                        nc.scalar.activation(
                                            out=ex[:, cb:sub * 512 + 512],
                                            in_=sc[:, cb:sub * 512 + 512],
                                            func=(mybir
                                                  .ActivationFunctionType.Exp),
                                            scale=SCALE,
                                        )
                                else:
                                    nc.scalar.activation(
                                        out=ex,
                                        in_=sc,
                                        func=mybir.ActivationFunctionType.Exp,
                                        scale=SCALE,
                                    )
                                if diag and mask_dve:
                                    # exact 0/1 triangle multiply on the
                                    # diagonal block (replaces PE mask MMs)
                                    for sub in range(pairw):
                                        j = kb0 + sub - 4 * qmb
                                        if j < 0:
                                            continue
                                        cb = sub * 512 + 128 * j
                                        with nc.allow_low_precision(
                                            reason="exact 0/1 mask"
                                        ):
                                            nc.vector.tensor_mul(
                                                ex[:, cb:cb + 128],
                                                ex[:, cb:cb + 128],
                                                trimask01,
                                            )
                                mms = []
                                for sub in range(pairw):
                                    kb = kb0 + sub
                                    j = kb - 4 * qmb
                                    col0 = 128 * j if j >= 0 else 0
                                    avw = QMB - col0
                                    mms.append((
                                        "sg", kb, col0, avw,
                                        ex[:, sub * 512 + col0:
                                            sub * 512 + col0 + avw],
                                    ))
                                avq.append((h, mms))
                            can_pop = qmb < 2 or blk > 0
                            if fill_first and filler and can_pop:
                                pop_filler()
                            while len(avq) > wave * lag:
                                emit_av(avq.pop(0))
                            if not fill_first and filler and can_pop:
                                pop_filler()
                        # drain + normalize: values = av[0:64] / av[64].
                        # Chunked over q so the recip->broadcast->mul links
                        # pipeline and the chain latency roughly halves.
                        def normalize(h):
                            av = avs[h]
                            dt_ = h // 2
                            pr = 64 * (h % 2)
                            # final wave: chunk so the chain pipelines and
                            # the PE-idle tail shrinks (elsewhere chunking
                            # just adds DVE queue pressure)
                            nchunks = norm_chunks
                            if qmb == N_QMB - 1 and w0 + wave >= HL:
                                nchunks = max(nchunks, 2)
                            cw = QMB // nchunks
                            for ci in range(nchunks):
                                c0 = ci * cw
                                rec = small.tile([1, QMB], F32R, tag="rec")
                                with nc.allow_low_precision(
                                    reason="softmax denom feeds bf16 matmul"
                                ):
                                    nc.vector.reciprocal(
                                        rec[:, 0:cw], av[64:65, c0:c0 + cw]
                                    )
                                rb = small.tile([64, QMB], F32R, tag="rb")
                                nc.gpsimd.partition_broadcast(
                                    rb[:, 0:cw], rec[:, 0:cw]
                                )
                                with nc.allow_low_precision(
                                    reason="attn values stored bf16"
                                ):
                                    nc.vector.tensor_mul(
                                        valuesT[pr:pr + 64, dt_,
                                                q0 + c0:q0 + c0 + cw],
                                        av[0:64, c0:c0 + cw],
                                        rb[:, 0:cw],
                                    )

                        rest = list(avq)
                        avq.clear()
                        for i, item in enumerate(rest):
                            emit_av(item)
                            h_done = item[0]
                            if not any(
                                it[0] == h_done for it in rest[i + 1:]
                            ):
                                normalize(h_done)
                    # ---- queue stage C for this qmb ----
                    if "C" not in stages:
                        continue
                    use_act = act_tail_copy and qmb == N_QMB - 1
                    for sti in range(QMB // 128):
                        filler.append(
                            ("C", make_c_item(qmb * 4 + sti, use_act))
                        )
                while filler:
                    filler.pop(0)[1]()

            if repeat == 1:
                if dma_interleave:
                    xts0 = load_xts(0, interleave_weights=True)
                    emit_bv2()
                    xts1 = load_xts(1)
                    load_const_weights(skip_interleaved=True)
                    body(0, xts0, xts1)
                else:
                    load_const_weights()
                    emit_bv2()
                    body(0)
            else:
                load_const_weights()
                emit_bv2()
                n_loop = repeat // unroll
                rem = repeat - n_loop * unroll
                if n_loop > 0:
                    with tc.For_i(
                        0, n_loop, 1,
                        hint_engines=(mybir.EngineType.PE,),
                        staggered_reset=staggered,
                    ) as it:
                        for _u in range(unroll):
                            body(it)
                for _u in range(rem):
                    body(0)
    nc.compile()
    return nc


def make_in_maps(x, W_qkv, b_qkv, W_out, b_out):
    """Host-side sharding: per-core input dict (bf16 weights/activations)."""
    x = np.asarray(x, dtype=np.float32)
    W_qkv = np.asarray(W_qkv, dtype=np.float32)
    b_qkv = np.asarray(b_qkv, dtype=np.float32)
    W_out = np.asarray(W_out, dtype=np.float32)
    in_maps = []
    xT_by_b = [
        np.ascontiguousarray(x[b_].T.astype(NP_BF16)) for b_ in range(B)
    ]
    for c in range(N_CORES):
        b_ = c // 4
        g = c % 4
        heads = [4 * g + i for i in range(HL)]
        # feature order: K(h0),K(h1),Q(h0),Q(h1),K(h2),K(h3),Q(h2),Q(h3)
        qk_cols = []
        for pair in range(2):
            h0, h1 = heads[2 * pair], heads[2 * pair + 1]
            for h_ in (h0, h1):
                base = h_ * 3 * HD + 1 * HD  # K
                qk_cols.extend(range(base, base + HD))
            for h_ in (h0, h1):
                base = h_ * 3 * HD + 0 * HD  # Q
                qk_cols.extend(range(base, base + HD))
        v_cols = []
        for h_ in heads:
            base = h_ * 3 * HD + 2 * HD  # V
            v_cols.extend(range(base, base + HD))
        qk_cols = np.array(qk_cols)
        v_cols = np.array(v_cols)
        in_maps.append({
            "xT": xT_by_b[b_],
            "wqk": np.ascontiguousarray(W_qkv[:, qk_cols].astype(NP_BF16)),
            "wv": np.ascontiguousarray(W_qkv[:, v_cols].astype(NP_BF16)),
            "wo": np.ascontiguousarray(
                W_out[g * FV:(g + 1) * FV, :].astype(NP_BF16)
            ),
            "bqk": np.ascontiguousarray(b_qkv[qk_cols].astype(np.float32)),
            "bv": np.ascontiguousarray(b_qkv[v_cols].astype(NP_BF16)),
        })
    return in_maps


_NC_CACHE = {}


def get_nc(repeat: int = 1, **kw):
    key = (repeat, tuple(sorted(kw.items())))
    if key not in _NC_CACHE:
        _NC_CACHE[key] = build_kernel(repeat, **kw)
    return _NC_CACHE[key]


def kernel(x, W_qkv, b_qkv, W_out, b_out):
    in_maps = make_in_maps(x, W_qkv, b_qkv, W_out, b_out)
    nc = get_nc(1)
    res = run_bass_kernel_spmd(nc, in_maps, list(range(N_CORES)))
    b_out = np.asarray(b_out, dtype=np.float32)
    out = np.zeros((B, S, D), dtype=np.float32)
    for b_ in range(B):
        acc = np.zeros((S, D), dtype=np.float32)
        for g in range(4):
            acc += np.asarray(res.results[4 * b_ + g]["out"], dtype=np.float32)
        out[b_] = acc + b_out[None, :]
    return out
